# revision 65
# baseline (speedup 1.0000x reference)
"""Trainium2 Bass kernel for DepthCueExtractor.

out[b,h,w,f] = mean_{a,c}(lfi[b,a,h,w,c]) * hv[b,h,f]
where hv[b,w,f] = colmean_h(f_maps[b,h,w,f]) / max_w(colmean), evaluated at w=h.

Sharding: 8 cores = (batch b in 0..3) x (h-half j in 0..1). Each core gets
  - lfi[b, :, 128j:128j+128, :, :]  (its h rows, f32, host-transposed [h,w,a,c])
  - f_maps[b] rolled by -128j along w (fp8 e4m3; its hv rows at w 0..127)
and computes out[b, 128j:128j+128, :, :] (stored bf16, widened on host).

Precision: fp8e4m3 f_maps + f32 everything-else + bf16 stores measures
1.39e-2 max rel err on the harness seed (gate 2e-2). lfi stays f32: the
min |mean_ac| is ~9e-8, so any absolute error in that sum blows up the
relative check. A pair-wise AllReduce(max) variant that halves the fmap
bytes was measured at ~30 us of collective latency on this runtime —
strictly worse than just loading the partner half (+6.7 us).

Schedule: all loads ride the sync HWDGE queue, fmap first (hv_n gates every
multiply), then lfi in tapered chunks so the post-load tail only owes the
last small chunk's reduce+mul. Column sums over the 256 h rows are a single
DoubleRow fp8 matmul pass (2 rows/cycle, h packed [128 partitions x 2]).
Stores ride the ACT ring. GpSimd (SWDGE) scatters hv rows onto partitions
and runs the seven early multiplies; DVE does all reduces, the max/normalize
chain, and the late (small) multiplies.
"""

import numpy as np
import ml_dtypes
from contextlib import ExitStack

import concourse.bass as bass
import concourse.bacc as bacc
import concourse.tile as tile
from concourse import mybir
from concourse.bass_utils import run_bass_kernel_spmd

F32 = mybir.dt.float32
BF16 = mybir.dt.bfloat16
F8 = mybir.dt.float8e4
B, A, H, W, C, F = 4, 9, 256, 256, 9, 64
HL = H // 2  # 128 h rows per core
N_CORES = 8

# lfi w-chunks: tapered so the tail after the last load is tiny; the first
# seven (GpSimd's mul share) are kept small since they serialize after hv_n
LFI_CHUNKS = [28, 28, 28, 28, 28, 28, 28, 24, 16, 12, 4, 4]
N_GPS_MULS = 8  # chunks [0..8) multiply on GpSimd (220 w): trace shows GPS
                # idles from 62 us while DVE's chunk-7 mul delays its reduces

FCHUNK = 1024           # fmap chunk: 16 w x 64 f (out-cols per chunk)
NF = (W * F) // FCHUNK  # 16

_PROGRAM_CACHE = {}


def build_program() -> bass.Bass:
    nc = bacc.Bacc("TRN2", target_bir_lowering=False, debug=False)
    lfi = nc.declare_dram_parameter("lfi", [HL, W, A, C], F32, isOutput=False)
    fmap = nc.declare_dram_parameter("fmap", [128, 2, W * F], F8, isOutput=False)
    outp = nc.declare_dram_parameter("out", [HL, W * F], BF16, isOutput=True)

    with ExitStack() as ctx:
        tc = ctx.enter_context(tile.TileContext(nc))
        const_pool = ctx.enter_context(tc.tile_pool(name="const", bufs=1))
        fpool = ctx.enter_context(tc.tile_pool(name="fmap", bufs=NF))
        ppool = ctx.enter_context(tc.tile_pool(name="psum", bufs=3, space="PSUM"))
        bpool = ctx.enter_context(tc.tile_pool(name="bcast", bufs=1, space="PSUM"))
        rpool = ctx.enter_context(tc.tile_pool(name="hvrow", bufs=4))
        hvpool = ctx.enter_context(tc.tile_pool(name="hv", bufs=1))
        lpool = ctx.enter_context(tc.tile_pool(name="lfi", bufs=8))
        mpool = ctx.enter_context(tc.tile_pool(name="m", bufs=len(LFI_CHUNKS)))
        opool = ctx.enter_context(tc.tile_pool(name="outp", bufs=8))

        # DoubleRow LDWEIGHTS wants weights as [Ki, Ko=2, M] with the dual-row
        # step a multiple of 16 elements, so pad the M stride to 16. memset
        # instead of a DMA keeps the first fmap load at the ring head.
        ones2 = const_pool.tile([128, 2, 16], F8)
        nc.vector.memset(ones2[:], 1.0)
        ones_col = const_pool.tile([1, 128], F32)
        nc.vector.memset(ones_col[:], 1.0)

        # ---- all load DMAs up front on the sync queue: fmap first ----
        fts = []
        for c in range(NF):
            ft = fpool.tile([128, 2, FCHUNK], F8)
            nc.sync.dma_start(out=ft[:], in_=fmap[:, :, FCHUNK * c : FCHUNK * (c + 1)])
            fts.append(ft)
        lts = []
        off = 0
        for wc in LFI_CHUNKS:
            lt = lpool.tile([128, wc, A, C], F32)
            nc.sync.dma_start(out=lt[:], in_=lfi[:, off : off + wc, :, :])
            lts.append((lt, off, wc))
            off += wc

        # ---- fmap column sums: one DoubleRow pass over all 256 h ----
        # hvw[w_local, f]: this core's 128 hv rows (w rolled, first half)
        hvw = hvpool.tile([128, F], F32)
        # every 32-w pair also lands in hv8 at base partition 0 with the pair
        # index on the free axis, so the 256-w max is one strided reduce
        hv8 = hvpool.tile([32, NF // 2, F], F32)
        for c in range(NF):
            pt = ppool.tile([1, FCHUNK], F32)
            for k in range(FCHUNK // 512):
                ks = slice(512 * k, 512 * (k + 1))
                nc.tensor.matmul(
                    pt[:, ks],
                    ones2[:, :, 0:1],
                    fts[c][:, :, ks],
                    start=True,
                    stop=True,
                    perf_mode=mybir.MatmulPerfMode.DoubleRow,
                )
            # PSUM->SBUF copies alternate ACT/DVE by pair (one engine's
            # cadence is slower than the PE DoubleRow pace); scatter per pair
            if c % 2 == 0:
                rowt = rpool.tile([1, 2 * FCHUNK], F32)
            dst = rowt[:, FCHUNK * (c % 2) : FCHUNK * (c % 2 + 1)]
            if (c // 2) % 2 == 0:
                nc.scalar.copy(dst, pt[:])
            else:
                nc.vector.tensor_scalar_mul(dst, pt[:], 1.0)
            if c % 2 == 1:
                a = c // 2
                nc.gpsimd.dma_start(
                    out=hv8[:, a, :],
                    in_=rowt[:].rearrange("p (w f) -> p w f", w=32),
                )
                if 16 * (c - 1) < 128:
                    nc.gpsimd.dma_start(
                        out=hvw[32 * a : 32 * (a + 1), :],
                        in_=rowt[:].rearrange("p (w f) -> p w f", w=32),
                    )

        # ---- lfi phase, with the max/normalize chain woven into DVE's
        # instruction stream after reduce 0 so DVE never idles ----
        hv_n = None
        for i, (lt, off, wc) in enumerate(lts):
            m_c = mpool.tile([128, wc], F32)
            nc.vector.reduce_sum(out=m_c[:], in_=lt[:], axis=mybir.AxisListType.XY)
            if i == 0:
                # max over the 8 pair groups (strided free-axis reduce), then
                # cross-partition max of [32, F] via 32x32 transposes
                run_max = hvpool.tile([32, F], F32)
                nc.vector.reduce_max(
                    out=run_max[:],
                    in_=hv8[:].rearrange("p a f -> p f a"),
                    axis=mybir.AxisListType.X,
                )
                hmT = hvpool.tile([F, 32], F32)
                for fj in range(F // 32):
                    nc.vector.transpose(
                        out=hmT[32 * fj : 32 * (fj + 1), 0:32],
                        in_=run_max[0:32, 32 * fj : 32 * (fj + 1)],
                    )
                mxc = hvpool.tile([F, 32], F32)
                nc.vector.memset(mxc[:], 0.0)
                nc.vector.reduce_max(
                    out=mxc[:, 0:1], in_=hmT[:], axis=mybir.AxisListType.X
                )
                mxr = hvpool.tile([32, F], F32)
                for pi in range(F // 32):
                    nc.vector.transpose(
                        out=mxr[0:32, 32 * pi : 32 * (pi + 1)],
                        in_=mxc[32 * pi : 32 * (pi + 1), 0:32],
                    )
                inv_row = hvpool.tile([1, F], F32)
                nc.vector.reciprocal(inv_row[:], mxr[0:1, :])
                inv_rep = bpool.tile([128, F], F32)
                nc.tensor.matmul(
                    inv_rep[:], ones_col[:], inv_row[:], start=True, stop=True
                )
                hv_n = hvpool.tile([128, F], F32)
                nc.vector.scalar_tensor_tensor(
                    out=hv_n[:],
                    in0=hvw[:],
                    scalar=1.0 / (A * C),
                    in1=inv_rep[:],
                    op0=mybir.AluOpType.mult,
                    op1=mybir.AluOpType.mult,
                )
            out_t = opool.tile([128, wc, F], BF16)
            eng = nc.gpsimd if i < N_GPS_MULS else nc.vector
            eng.tensor_tensor(
                out=out_t[:],
                in0=m_c[:].unsqueeze(2).broadcast_to([128, wc, F]),
                in1=hv_n[:].unsqueeze(1).broadcast_to([128, wc, F]),
                op=mybir.AluOpType.mult,
            )
            # stores ride the ACT ring so a gated store never blocks a load
            nc.scalar.dma_start(
                out=outp[:, F * off : F * (off + wc)],
                in_=out_t.rearrange("p w f -> p (w f)"),
            )

    nc.compile()
    return nc


def _get_program() -> bass.Bass:
    if "nc" not in _PROGRAM_CACHE:
        _PROGRAM_CACHE["nc"] = build_program()
    return _PROGRAM_CACHE["nc"]


def make_in_maps(lfi: np.ndarray, f_maps: np.ndarray) -> list[dict]:
    f8 = ml_dtypes.float8_e4m3
    in_maps = []
    for core in range(N_CORES):
        b, j = divmod(core, 2)
        lfi_s = np.ascontiguousarray(
            lfi[b, :, HL * j : HL * (j + 1), :, :].transpose(1, 2, 0, 3)
        )
        # roll so local hv rows are w 0..127, then pack h as [128 p, 2 hh]
        fm = np.roll(f_maps[b], -HL * j, axis=1)        # [256 h, 256 w, 64 f]
        fm = fm.reshape(2, 128, W * F).transpose(1, 0, 2)  # [128 p, 2 hh, (w f)]
        in_maps.append(
            {
                "lfi": lfi_s,
                "fmap": np.ascontiguousarray(fm.astype(f8)),
            }
        )
    return in_maps


def assemble_out(results: list[dict]) -> np.ndarray:
    out = np.empty((B, H, W, F), np.float32)
    for core in range(N_CORES):
        b, j = divmod(core, 2)
        out[b, HL * j : HL * (j + 1)] = (
            results[core]["out"].astype(np.float32).reshape(HL, W, F)
        )
    return out


def kernel(lfi: np.ndarray, f_maps: np.ndarray) -> np.ndarray:
    lfi = np.asarray(lfi, dtype=np.float32)
    f_maps = np.asarray(f_maps, dtype=np.float32)
    nc = _get_program()
    in_maps = make_in_maps(lfi, f_maps)
    res = run_bass_kernel_spmd(nc, in_maps, list(range(N_CORES))).results
    return assemble_out(res)
